# revision 2
# baseline (speedup 1.0000x reference)
"""Conv2d(128->256, k=3, s=1, VALID) on 8 TRN2 NeuronCores.

Strategy: data-parallel over batch (32 images -> 4 per core). On each core
the conv runs as 1-D Winograd F(2,3) along H, cutting PE work 1.5x vs the
9-tap direct form:

  per pair of output rows (2t, 2t+1):
    V0 = x[2t] - x[2t+2]    V1 = x[2t+1] + x[2t+2]      (DVE, bf16, 2x mode)
    V2 = x[2t+2] - x[2t+1]  V3 = x[2t+1] - x[2t+3]
    M_u = sum_kw U[u,kw]^T @ V_u[:, ow+kw]              (PE, bf16, PSUM accum)
    y[2t]   = M0 + M1 + M2  (+bias)                     (DVE TT + STT)
    y[2t+1] = M1 - M2 - M3  (+bias)
  with U[u,kw] = sum_kh G[u,kh] w[:,:,kh,kw] precomputed on host.

Work per (och, 4-tile block): 12 matmuls of 440 moving elems (vs 18 for the
direct form). The scalar engine copies M from PSUM to SBUF as bf16 so the
inverse-transform tensor_tensor ops hit the DVE 2x packing mode; the final
adds write f32 directly. V is computed two blocks ahead of the PE.
"""

import numpy as np

import concourse.bass as bass
from concourse import bacc
import concourse.mybir as mybir
import concourse.tile as tile
from concourse.bass_utils import run_bass_kernel_spmd

N_CORES = 8
N, IC, H, W = 32, 128, 112, 112
OC, K = 256, 3
OH, OW = H - K + 1, W - K + 1  # 110, 110
NPC = N // N_CORES  # images per core
OCH = OC // 128  # oc halves
HT = H // 2  # 56 row-pairs

_f32 = mybir.dt.float32
_bf16 = mybir.dt.bfloat16

# t-blocks: 13 blocks of 4 tiles + 1 block of 3 (55 tiles = 13*4 + 3)
T_BLOCKS = [(i * 4, 4) for i in range(13)] + [(52, 3)]
NBLK = len(T_BLOCKS)

_ADD = mybir.AluOpType.add
_SUB = mybir.AluOpType.subtract


def _build_program(npc: int = NPC, zero_bias: bool = True) -> bacc.Bacc:
    nc = bacc.Bacc("TRN2", target_bir_lowering=False, debug=False)
    xd = nc.dram_tensor("x", [npc, IC, H * W], _bf16, kind="ExternalInput").ap()
    wd = nc.dram_tensor("w", [IC, 4 * K * OCH * 128], _bf16, kind="ExternalInput").ap()
    bd = nc.dram_tensor("b", [128, OCH], _f32, kind="ExternalInput").ap()
    od = nc.dram_tensor("out", [NPC, OC, OH, OW], _bf16, kind="ExternalOutput").ap()

    with tile.TileContext(nc) as tc:
        with (
            tc.tile_pool(name="wp", bufs=1) as wp,
            tc.tile_pool(name="xp", bufs=2) as xp,
            tc.tile_pool(name="vp", bufs=4) as vp,
            tc.tile_pool(name="mp", bufs=3) as mp,
            tc.tile_pool(name="tp", bufs=3) as tp,
            tc.tile_pool(name="yp", bufs=3) as yp,
            tc.tile_pool(name="pp", bufs=2, space="PSUM") as pp,
        ):
            w_sb = wp.tile([128, 4 * K * OCH * 128], _bf16)
            nc.sync.dma_start(w_sb[:], wd[:])
            wv = w_sb[:].rearrange("p (u k o c) -> p u k o c", u=4, k=K, o=OCH)
            b_sb = wp.tile([128, OCH], _f32)
            nc.sync.dma_start(b_sb[:], bd[:])
            # absorb the weight-DMA semaphore into the PE clock so real
            # matmuls never spend a wait slot on it
            nc.tensor.ldweights(w_sb[:, :128])

            def load_image(n):
                xt = xp.tile([128, H * W], _bf16)
                for c in range(4):
                    nc.sync.dma_start(
                        xt[:, c * 28 * W : (c + 1) * 28 * W],
                        xd[n, :, c * 28 * W : (c + 1) * 28 * W],
                    )
                return xt

            def make_v(xt, t0, tb):
                """V tile [128, 4u, 4t, 112] for tiles t0..t0+tb-1."""
                v = vp.tile([128, 4, 4, W], _bf16)
                # x rows 2t+i: view as [parity, row-pair, w]
                xv = xt[:].rearrange("p (t two w) -> p two t w", two=2, w=W)
                d0 = xv[:, 0, t0 : t0 + tb, :]  # rows 2t
                d1 = xv[:, 1, t0 : t0 + tb, :]  # rows 2t+1
                d2 = xv[:, 0, t0 + 1 : t0 + 1 + tb, :]  # rows 2t+2
                d3 = xv[:, 1, t0 + 1 : t0 + 1 + tb, :]  # rows 2t+3
                nc.vector.tensor_sub(v[:, 0, :tb, :], d0, d2)
                nc.vector.tensor_add(v[:, 1, :tb, :], d1, d2)
                nc.vector.tensor_sub(v[:, 2, :tb, :], d2, d1)
                nc.vector.tensor_sub(v[:, 3, :tb, :], d1, d3)
                return v

            blocks = [(n, t0, tb) for n in range(npc) for (t0, tb) in T_BLOCKS]

            x_tiles = [load_image(0)]
            v_tiles = {
                0: make_v(x_tiles[0], *T_BLOCKS[0]),
                1: make_v(x_tiles[0], *T_BLOCKS[1]),
            }

            for g, (n, t0, tb) in enumerate(blocks):
                blk_in_img = g % NBLK
                if blk_in_img == 0 and g + NBLK < len(blocks):
                    x_tiles.append(load_image((n + 1) % npc))
                v = v_tiles.pop(g)
                fd = tb * OW

                m = mp.tile([128, 4, OCH, 4 * OW], _bf16)
                for och in range(OCH):
                    ps = pp.tile([128, 4, 512], _f32)
                    for u in range(4):
                        for kw in range(K):
                            nc.tensor.matmul(
                                ps[:, u, :fd],
                                lhsT=wv[:, u, kw, och, :],
                                rhs=v[:, u, :tb, kw : kw + OW],
                                start=(kw == 0),
                                stop=(kw == K - 1),
                            )
                    nc.scalar.activation(
                        m[:, :, och, :fd],
                        ps[:, :, :fd],
                        mybir.ActivationFunctionType.Copy,
                    )

                # V for block g+2 keeps the DVE ahead of the PE
                if g + 2 < len(blocks):
                    _, nt0, ntb = blocks[g + 2]
                    delta = (g + 2) // NBLK - g // NBLK
                    v_tiles[g + 2] = make_v(x_tiles[delta], nt0, ntb)

                # inverse transform. y layout [och, t, p, w] keeps the
                # out-DMA source contiguous; for full blocks the och stride
                # (880) equals 4 t-strides (220), so (och, t) merges into a
                # single uniform dim and both halves go in one DVE op.
                tmp = tp.tile([128, 2, OCH * 4 * OW], _bf16)
                # bf16 output: DVE runs the final adds at 2x rate and the
                # out-DMA halves; the host upconverts to f32 after gather.
                y = yp.tile([128, OCH, 4, 2, OW], _bf16)
                mflat = m[:].rearrange("p u o f -> p u (o f)")
                if tb == 4:
                    nc.vector.tensor_add(tmp[:, 0, :], mflat[:, 0], mflat[:, 1])
                    nc.vector.tensor_sub(tmp[:, 1, :], mflat[:, 1], mflat[:, 2])
                else:
                    for och in range(OCH):
                        nc.vector.tensor_add(
                            tmp[:, 0, och * 4 * OW :][:, :fd],
                            m[:, 0, och, :fd],
                            m[:, 1, och, :fd],
                        )
                        nc.vector.tensor_sub(
                            tmp[:, 1, och * 4 * OW :][:, :fd],
                            m[:, 1, och, :fd],
                            m[:, 2, och, :fd],
                        )

                def rows(ap1):  # [128, OCH*4*OW] -> [128, OCH*4, OW]
                    return ap1.rearrange("p (r w) -> p r w", w=OW)

                ymrg = y[:].rearrange("p o t two w -> p (o t) two w")
                if zero_bias and tb == 4:
                    nc.vector.tensor_add(
                        ymrg[:, :, 0, :], rows(tmp[:, 0]), rows(mflat[:, 2])
                    )
                    nc.vector.tensor_sub(
                        ymrg[:, :, 1, :], rows(tmp[:, 1]), rows(mflat[:, 3])
                    )
                else:
                    for och in range(OCH):
                        bb = 0.0 if zero_bias else b_sb[:, och : och + 1]
                        t0c = tmp[:, 0, och * 4 * OW :][:, :fd].rearrange(
                            "p (t w) -> p t w", w=OW
                        )
                        t1c = tmp[:, 1, och * 4 * OW :][:, :fd].rearrange(
                            "p (t w) -> p t w", w=OW
                        )
                        m2c = m[:, 2, och, :fd].rearrange("p (t w) -> p t w", w=OW)
                        m3c = m[:, 3, och, :fd].rearrange("p (t w) -> p t w", w=OW)
                        nc.vector.scalar_tensor_tensor(
                            y[:, och, :tb, 0, :], t0c, bb, m2c, _ADD, _ADD
                        )
                        nc.vector.scalar_tensor_tensor(
                            y[:, och, :tb, 1, :], t1c, bb, m3c, _ADD, _SUB
                        )

                for och in range(OCH):
                    nc.sync.dma_start(
                        od[n, och * 128 : (och + 1) * 128, 2 * t0 : 2 * (t0 + tb), :],
                        y[:, och, :tb, :, :],
                    )

                if blk_in_img == NBLK - 1 and len(x_tiles) > 1:
                    x_tiles.pop(0)
    return nc


def _prep_in_maps(x, weight, bias):
    import ml_dtypes

    bf = ml_dtypes.bfloat16
    x = np.asarray(x, dtype=np.float32)
    weight = np.asarray(weight, dtype=np.float32)
    bias = np.asarray(bias, dtype=np.float32)

    # U[u, kw, oc, ic] = sum_kh G[u,kh] w[oc,ic,kh,kw], laid out as
    # [ic, (u, kw, och, oc_in_half)]
    G = np.array(
        [[1, 0, 0], [0.5, 0.5, 0.5], [0.5, -0.5, 0.5], [0, 0, 1]], dtype=np.float64
    )
    U = np.einsum("uk,oikw->uwoi", G, weight.astype(np.float64))  # [4, K, OC, IC]
    wt = np.ascontiguousarray(U.transpose(3, 0, 1, 2).reshape(IC, 4 * K * OC).astype(bf))
    b2 = np.ascontiguousarray(bias.reshape(OCH, 128).T)
    xb = np.ascontiguousarray(x.astype(bf).reshape(N, IC, H * W))
    return [
        {
            "x": np.ascontiguousarray(xb[c * NPC : (c + 1) * NPC]),
            "w": wt,
            "b": b2,
        }
        for c in range(N_CORES)
    ]


def kernel(x: np.ndarray, weight: np.ndarray, bias: np.ndarray) -> np.ndarray:
    zero_bias = not np.any(np.asarray(bias))
    nc = _build_program(zero_bias=zero_bias)
    if not nc.is_finalized():
        nc.finalize()
    in_maps = _prep_in_maps(x, weight, bias)
    res = run_bass_kernel_spmd(nc, in_maps, list(range(N_CORES)))
    out = np.concatenate([res.results[c]["out"] for c in range(N_CORES)], axis=0)
    return np.asarray(out, dtype=np.float32)
